# revision 1
# baseline (speedup 1.0000x reference)
"""DNC memory-controller step (nn_Controller_85332410237553) on 8 trn2 cores.

Data-parallel: core k handles batches 4k..4k+3. Heavy op is streaming
link [b,1,1024,1024] through two small matmuls (rw @ link_new and
rw @ link_new^T) which decompose algebraically onto raw `link`:
  bwd[r,j] = (rwL - wrwL)[r,j] - w_j*rwL[r,j] + (rw.w)*p_j - rw[r,j]*w_j*p_j
  fwd[r,i] = (1-w_i)*Lrw[r,i] - Lwrw[r,i] + w_i*(rw.p) - rw[r,i]*w_i*p_i
so link is read once, streamed through PE (natural orientation for bwd,
PE-transposed 128x128 blocks for fwd).
"""
import os
from contextlib import ExitStack

import numpy as np

B, C, D, R, NW = 32, 1024, 64, 4, 1
NC = 8          # cores
BB = B // NC    # batches per core = 4
K = C // 128    # 8 c-chunks
EPS = 1e-6
SIZES = [NW*D, NW*D, R, NW, NW, R*(1+2*NW), NW*D, NW, R*D, R, 1, R*D, NW*D, NW*R]
OFFS = np.cumsum([0] + SIZES)
HX, HW_IF = 512, 796

# ---- scalar-column indices in S_pre [4(b), NS] ------------------------
# order matters: ms cols contiguous (r), pien cols contiguous (r,m)
S_FG = 0            # 4: sigmoid free_gate r=0..3
S_GAG = 4           # 1: ga*gw
S_GAG2 = 5          # 1: (1-ga)*gw
S_WS = 6            # 1: softplus write_strength
S_RS = 7            # 4: softplus read_strengths
S_AS = 11           # 1: softplus alloc_strength
S_NAS = 12          # 1: -softplus alloc_strength
S_MS = 13           # 4: softplus mode_strengths r
S_PIE = 17          # 12: normalized read_mode (r,m) m in (bwd,cnt,fwd)
S_KNW = 29          # 1: ||wmask*wkey||
S_KNR = 30          # 4: ||rmask_r*rkey_r||
NS = 34

def build_program():
    import concourse.bass as bass
    import concourse.bacc as bacc
    import concourse.mybir as mybir
    import concourse.tile as tile

    dt = mybir.dt
    f32 = dt.float32
    f32r = dt.float32r
    AF = mybir.ActivationFunctionType
    ALU = mybir.AluOpType
    AX = mybir.AxisListType
    AP = bass.AP

    nc = bacc.Bacc("TRN2", target_bir_lowering=False, debug=False,
                   num_devices=NC)

    # ---- dram io ----
    din = {}
    def dram_in(name, shape):
        din[name] = nc.dram_tensor(name, list(shape), f32, kind="ExternalInput").ap()
    dram_in("x", (BB, HX))
    dram_in("W_if", (HX, HW_IF))
    dram_in("b_if", (HW_IF,))
    dram_in("memory", (BB, C, D))
    dram_in("usage", (BB, C))
    din["link"] = nc.dram_tensor("link", [BB, C, C], dt.float32r,
                                 kind="ExternalInput").ap()
    dram_in("precedence", (BB, C))
    dram_in("prw", (BB, R, C))
    dram_in("pww", (BB, C))
    dram_in("eye128", (128, 128))
    dram_in("ones128", (128, 128))
    dram_in("selb", (BB, BB * 128))
    din["eyer"] = nc.dram_tensor("eyer", [128, 128], dt.float32r,
                                 kind="ExternalInput").ap()
    out_rv = nc.dram_tensor("read_vectors", [BB, R, D], f32,
                            kind="ExternalOutput").ap()

    def bc(ap_, newap):
        # manual broadcast/strided AP on same tensor
        return AP(ap_.tensor, ap_.offset, newap)

    with tile.TileContext(nc) as tc:
        with ExitStack() as ctx:
            emit(ctx, tc, nc, din, out_rv, bass, mybir, tile)
    nc.compile()
    return nc


def emit(ctx, tc, nc, din, out_rv, bass, mybir, tile):
    dt = mybir.dt
    f32, f32r = dt.float32, dt.float32r
    AF = mybir.ActivationFunctionType
    ALU = mybir.AluOpType
    AX = mybir.AxisListType
    AP = bass.AP
    V, S, T, SY, GP = nc.vector, nc.scalar, nc.tensor, nc.sync, nc.gpsimd

    def _ap(base):
        return base if isinstance(base, AP) else base[:]

    def A(base, off, dims):
        # keep base partition dim, replace free dims
        b = _ap(base)
        return AP(b.tensor, b.offset + off, [list(b.ap[0])] + dims)

    def M(base, off, dims):
        # fully manual AP (dims[0] is the partition dim)
        b = _ap(base)
        return AP(b.tensor, b.offset + off, dims)

    pc = ctx.enter_context(tc.tile_pool(name="const", bufs=1))
    pp = ctx.enter_context(tc.tile_pool(name="prep", bufs=1))
    plink = ctx.enter_context(tc.tile_pool(name="link", bufs=2))
    plt = ctx.enter_context(tc.tile_pool(name="lt", bufs=1))
    pmem = ctx.enter_context(tc.tile_pool(name="mem", bufs=2))
    pw = ctx.enter_context(tc.tile_pool(name="work", bufs=1))
    psc = ctx.enter_context(tc.tile_pool(name="scratch", bufs=4))
    ptp = ctx.enter_context(tc.tile_pool(name="tpsum", bufs=1, space="PSUM"))
    pacc = ctx.enter_context(tc.tile_pool(name="laccpsum", bufs=1, space="PSUM"))
    psmall = ctx.enter_context(tc.tile_pool(name="spsum", bufs=1, space="PSUM"))

    def pet(in_ap, np_, nf_):
        # PE transpose [np_, nf_] -> psum [nf_, np_]
        ps = ptp.tile([nf_, np_], f32, tag="tp", bufs=2)
        T.transpose(ps[:], in_ap, eye[:np_, :np_])
        return ps

    # ---------------- constants / weights ----------------
    eye_t = pc.tile([128, 128], f32); SY.dma_start(eye_t[:], din["eye128"])
    eye = eye_t[:]
    ones_t = pc.tile([128, 128], f32); SY.dma_start(ones_t[:], din["ones128"])
    ones = ones_t[:]
    eyer_t = pc.tile([128, 128], f32r); SY.dma_start(eyer_t[:], din["eyer"])
    eyer = eyer_t[:]
    W_sb = pc.tile([128, 4 * HW_IF], f32)
    for k in range(4):
        SY.dma_start(W_sb[:, k*HW_IF:(k+1)*HW_IF],
                     M(din["W_if"], k*128*HW_IF, [[HW_IF, 128], [1, HW_IF]]))
    b_rep = pc.tile([BB, HW_IF], f32)
    for bb in range(BB):
        SY.dma_start(b_rep[bb:bb+1, :], M(din["b_if"], 0, [[HW_IF, 1], [1, HW_IF]]))

    # ---------------- interface projection y = x@W + b ----------------
    x_nat = pp.tile([BB, HX], f32); SY.dma_start(x_nat[:], din["x"])
    xT = pp.tile([128, 4 * BB], f32)
    for k in range(4):
        ps = pet(x_nat[:, k*128:(k+1)*128], BB, 128)
        S.copy(xT[:, k*BB:(k+1)*BB], ps[:])
    y_ps0 = psmall.tile([BB, 512], f32, tag="sp")
    y_ps1 = psmall.tile([BB, HW_IF - 512], f32, tag="sp")
    for k in range(4):
        T.matmul(y_ps0[:], xT[:, k*BB:(k+1)*BB],
                 W_sb[:, k*HW_IF:k*HW_IF+512],
                 start=(k == 0), stop=(k == 3))
        T.matmul(y_ps1[:], xT[:, k*BB:(k+1)*BB],
                 W_sb[:, k*HW_IF+512:(k+1)*HW_IF],
                 start=(k == 0), stop=(k == 3))
    y = pp.tile([BB, HW_IF], f32)
    V.tensor_add(y[:, 0:512], y_ps0[:], b_rep[:, 0:512])
    V.tensor_add(y[:, 512:HW_IF], y_ps1[:], b_rep[:, 512:HW_IF])

    # ---------------- heads ----------------
    sig1 = pp.tile([BB, 70], f32)   # ev(64) fg(4) ga gw  <- y[64:134]
    S.activation(sig1[:], y[:, 64:134], AF.Sigmoid)
    sig2 = pp.tile([BB, 320], f32)  # rmask(256) wmask(64) <- y[472:792]
    S.activation(sig2[:], y[:, 472:792], AF.Sigmoid)
    ev = sig1[:, 0:64]
    rmask, wmask = sig2[:, 0:256], sig2[:, 256:320]
    wv = y[:, 0:64]
    wkey, rkey = y[:, 146:210], y[:, 211:467]

    S_pre = pp.tile([BB, NS], f32)
    S.copy(S_pre[:, S_FG:S_FG+4], sig1[:, 64:68])
    V.tensor_mul(S_pre[:, S_GAG:S_GAG+1], sig1[:, 68:69], sig1[:, 69:70])
    V.tensor_sub(S_pre[:, S_GAG2:S_GAG2+1], sig1[:, 69:70], S_pre[:, S_GAG:S_GAG+1])
    spx = pp.tile([BB, 10], f32)   # softplus = ln(1 + exp(x))
    S.activation(spx[:, 0:1], y[:, 210:211], AF.Exp)
    S.activation(spx[:, 1:6], y[:, 467:472], AF.Exp)
    S.activation(spx[:, 6:10], y[:, 792:796], AF.Exp)
    S.activation(S_pre[:, S_WS:S_WS+1], spx[:, 0:1], AF.Ln, bias=1.0)
    S.activation(S_pre[:, S_RS:S_RS+5], spx[:, 1:6], AF.Ln, bias=1.0)
    S.activation(S_pre[:, S_MS:S_MS+4], spx[:, 6:10], AF.Ln, bias=1.0)
    V.tensor_scalar_mul(S_pre[:, S_NAS:S_NAS+1], S_pre[:, S_AS:S_AS+1], -1.0)
    # read mode softmax (unnorm exp * recip-sum)
    pie = pp.tile([BB, 12], f32)
    S.activation(pie[:], y[:, 134:146], AF.Exp)
    piZ = pp.tile([BB, 4], f32)
    V.tensor_reduce(piZ[:], A(pie, 0, [[3, 4], [1, 3]]), AX.X, ALU.add)
    piZr = pp.tile([BB, 4], f32)
    V.reciprocal(piZr[:], piZ[:])
    for m in range(3):
        V.tensor_tensor(A(S_pre, S_PIE+m, [[3, 4]]), A(pie, m, [[3, 4]]),
                        piZr[:], ALU.mult)
    # masked keys + norms
    wmk = pp.tile([BB, 64], f32);  V.tensor_mul(wmk[:], wmask, wkey)
    wmk2 = pp.tile([BB, 64], f32); V.tensor_mul(wmk2[:], wmk[:], wmask)
    wm2 = pp.tile([BB, 64], f32);  V.tensor_mul(wm2[:], wmask, wmask)
    rmk = pp.tile([BB, 256], f32); V.tensor_mul(rmk[:], rmask, rkey)
    rmk2 = pp.tile([BB, 256], f32); V.tensor_mul(rmk2[:], rmk[:], rmask)
    rm2 = pp.tile([BB, 256], f32); V.tensor_mul(rm2[:], rmask, rmask)
    sq = pp.tile([BB, 256], f32)
    V.tensor_mul(sq[:, 0:64], wmk[:], wmk[:])
    kn2 = pp.tile([BB, 5], f32)
    V.tensor_reduce(kn2[:, 0:1], sq[:, 0:64], AX.X, ALU.add)
    V.tensor_mul(sq[:], rmk[:], rmk[:])
    V.tensor_reduce(kn2[:, 1:5], A(sq, 0, [[64, 4], [1, 64]]), AX.X, ALU.add)
    S.activation(kn2[:], kn2[:], AF.Ln)
    S.activation(S_pre[:, S_KNW:S_KNW+5], kn2[:], AF.Exp, scale=0.5)  # sqrt

    # replicated scalars SREP_b[128, NS] via 0-step dma broadcast
    selb_t = pc.tile([BB, BB * 128], f32)
    SY.dma_start(selb_t[:], din["selb"])
    ROWS_pre = pp.tile([BB, 128], f32)
    S.copy(ROWS_pre[:, 0:64], ev)
    S.copy(ROWS_pre[:, 64:128], wv)
    SREP, ROWR = [], []
    for b in range(BB):
        psr = psmall.tile([128, NS], f32, tag="sp", name=f"psrep{b}")
        T.matmul(psr[:], selb_t[:, b*128:(b+1)*128], S_pre[:],
                 start=True, stop=True)
        t = pp.tile([128, NS], f32, tag=f"srep{b}", name=f"srep{b}")
        S.copy(t[:], psr[:])
        SREP.append(t)
        psr2 = psmall.tile([128, 128], f32, tag="sp", name=f"prowr{b}")
        T.matmul(psr2[:], selb_t[:, b*128:(b+1)*128], ROWS_pre[:],
                 start=True, stop=True)
        t2 = pp.tile([128, 128], f32, tag=f"rowr{b}", name=f"rowr{b}")
        V.tensor_copy(t2[:], psr2[:])
        ROWR.append(t2)
    def scol(b, c):
        return SREP[b][:, c:c+1]

    # proj-matmul rhs: [128, 2] per b (stacked d: rows 0:64 key-col, 64:128 mask2-col)
    RHSW = pw.tile([128, 2 * BB], f32)
    V.memset(RHSW[:], 0.0)
    ps = pet(wmk2[:], BB, 64)       # [64, BB]
    S.copy(M(RHSW, 0, [[2*BB, 64], [2, BB]]), ps[:])
    ps = pet(wm2[:], BB, 64)
    S.copy(M(RHSW, 64*2*BB + 1, [[2*BB, 64], [2, BB]]), ps[:])
    RHSR = pw.tile([128, 8 * BB], f32)
    V.memset(RHSR[:], 0.0)
    for r in range(R):
        ps = pet(rmk2[:, r*64:(r+1)*64], BB, 64)
        S.copy(M(RHSR, r, [[8*BB, 64], [8, BB]]), ps[:])
        ps = pet(rm2[:, r*64:(r+1)*64], BB, 64)
        S.copy(M(RHSR, 64*8*BB + 4 + r, [[8*BB, 64], [8, BB]]), ps[:])

    # ---------------- cP loads: usage/pww/precedence/prw ----------------
    def load_cp32(name):
        nat = pp.tile([BB, C], f32, tag="nat", name=f"nat_{name}", bufs=2)
        SY.dma_start(nat[:], din[name])
        t = pw.tile([128, 32], f32, tag=f"cp_{name}")
        for k in range(K):
            ps = pet(nat[:, k*128:(k+1)*128], BB, 128)
            S.copy(M(t, k, [[32, 128], [8, BB]]), ps[:])
        return t
    epsc = pw.tile([128, 1], f32, tag="epsc", name="epsc")
    V.memset(epsc[:], EPS)
    u32 = load_cp32("usage")
    pw32 = load_cp32("pww")
    p32 = load_cp32("precedence")
    rw_nat = pp.tile([BB*R, C], f32)
    SY.dma_start(rw_nat[:], M(din["prw"], 0, [[C, BB*R], [1, C]]))
    rwT = pw.tile([128, 128], f32)   # free = r*32 + b*8 + k
    for k in range(K):
        ps = pet(rw_nat[:, k*128:(k+1)*128], BB*R, 128)  # [128, 16(b,r)]
        for r in range(R):
            S.copy(M(rwT, r*32 + k, [[128, 128], [8, BB]]),
                   M(ps, r, [[16, 128], [4, BB]]))

    # ---------------- usage update + allocation (batched cP) -------------
    t1 = psc.tile([128, 32], f32, tag="t1")
    u1 = pw.tile([128, 32], f32)
    V.tensor_mul(t1[:], u32[:], pw32[:])
    V.tensor_add(u1[:], u32[:], pw32[:])
    V.tensor_sub(u1[:], u1[:], t1[:])
    M4 = pw.tile([128, 128], f32)   # (rw*fg - 1) in rbk layout
    for r in range(R):
        for b in range(BB):
            V.tensor_scalar(M4[:, r*32+b*8:r*32+(b+1)*8],
                            rwT[:, r*32+b*8:r*32+(b+1)*8],
                            scol(b, S_FG+r), 1.0, ALU.mult, ALU.subtract)
    phi = pw.tile([128, 32], f32)
    V.tensor_mul(t1[:], M4[:, 0:32], M4[:, 32:64])
    V.tensor_mul(phi[:], M4[:, 64:96], M4[:, 96:128])
    V.tensor_mul(phi[:], phi[:], t1[:])
    u2 = pw.tile([128, 32], f32)
    V.tensor_mul(u2[:], u1[:], phi[:])

    # p-dots (rw.p) partials -> PARTSD cols b*8+r  (w-dots later: b*8+4+r)
    PARTSD = pw.tile([128, 32], f32)
    scr = psc.tile([128, 8], f32, tag="scr", bufs=2)
    for b in range(BB):
        for r in range(R):
            scr = psc.tile([128, 8], f32, tag="scr", bufs=2)
            V.tensor_mul(scr[:], rwT[:, r*32+b*8:r*32+(b+1)*8],
                         p32[:, b*8:(b+1)*8])
            V.tensor_reduce(PARTSD[:, b*8+r:b*8+r+1], scr[:], AX.X, ALU.add)

    # ============== per-batch heavy pipeline ==============
    LN_J = C  # one i_hi row-block is [128, C]
    for b in range(BB):
        # -- link load + transpose --
        LN = []
        for ih in range(K):
            t = plink.tile([128, C], f32r, tag=f"lnat{ih}", name=f"ln{b}_{ih}")
            SY.dma_start(t[:], M(din["link"], b*C*C + ih*128*C,
                                 [[C, 128], [1, C]]))
            LN.append(t)
        LT = [plt.tile([128, C], f32r, tag=f"lt{jh}", name=f"lt{b}_{jh}")
              for jh in range(K)]
        for ih in range(K):
            for jh in range(K):
                ps = ptp.tile([128, 128], f32r, tag="tpr", name=f"tpr{b}_{ih}_{jh}", bufs=3)
                T.transpose(ps[:], LN[ih][:, jh*128:(jh+1)*128], eyer)
                dst = LT[jh][:, ih*128:(ih+1)*128]
                if (ih + jh) % 2 == 0:
                    V.tensor_copy(dst, ps[:])
                else:
                    S.copy(dst, ps[:])

        # -- memory load + MT stack --
        Mb = pmem.tile([128, K*D], f32, tag="m", name=f"m{b}")
        SY.dma_start(Mb[:], M(din["memory"], b*C*D,
                              [[D, 128], [128*D, K], [1, D]]))
        MTS = pmem.tile([128, C], f32, tag="mts", name=f"mts{b}")
        for k in range(K):
            ps = pet(Mb[:, k*D:(k+1)*D], 128, D)    # [64, 128]
            V.tensor_copy(MTS[0:64, k*128:(k+1)*128], ps[:])
            S.square(M(MTS, 64*C + k*128, [[C, 64], [1, 128]]), ps[:])
        # -- write-content proj + norm (fp32, tiny ap) --
        WPN = pw.tile([128, 64], f32, tag="wpn", bufs=2)  # col = t*32 + b*8 + k
        for k in range(K):
            psW = psmall.tile([128, 2], f32, tag="sp")
            T.matmul(psW[:], MTS[:, k*128:(k+1)*128],
                     RHSW[:, b*2:(b+1)*2], start=True, stop=True)
            V.tensor_copy(M(WPN, b*8 + k, [[64, 128], [32, 2]]), psW[:])
        # cosine-w: e_w = exp(sp_ws * proj / (knw*mn + eps)) ; accum Z
        PARTS1 = pw.tile([128, 2], f32, tag="parts1", bufs=2)
        mnw = psc.tile([128, 8], f32, tag="mnw", bufs=2)
        S.activation(mnw[:], WPN[:, 32 + b*8:32 + (b+1)*8], AF.Ln)
        S.activation(mnw[:], mnw[:], AF.Exp, scale=0.5)
        V.tensor_scalar(mnw[:], mnw[:], scol(b, S_KNW), EPS, ALU.mult, ALU.add)
        dnr = psc.tile([128, 8], f32, tag="dnr", bufs=2)
        V.reciprocal(dnr[:], mnw[:])
        simw = psc.tile([128, 8], f32, tag="simw", bufs=2)
        V.tensor_mul(simw[:], WPN[:, b*8:(b+1)*8], dnr[:])
        ew = pw.tile([128, 8], f32, tag="ew", bufs=2)
        S.activation(ew[:], simw[:], AF.Exp, scale=scol(b, S_WS),
                     accum_out=PARTS1[:, 1:2])
        # alloc: e_a = exp(sp_as*(1-u2)) ; accum Z
        ea = pw.tile([128, 8], f32, tag="ea", bufs=2)
        S.activation(ea[:], u2[:, b*8:(b+1)*8], AF.Exp,
                     bias=scol(b, S_AS), scale=scol(b, S_NAS),
                     accum_out=PARTS1[:, 0:1])
        psZ = psmall.tile([128, 2], f32, tag="sp")
        T.matmul(psZ[:], ones[:, 0:128], PARTS1[:], start=True, stop=True)
        zrec1 = psc.tile([128, 2], f32, tag="zrec1", bufs=2)
        V.reciprocal(zrec1[:], psZ[:])
        w32b = pw.tile([128, 8], f32, tag=f"w32_{b}")
        ta = psc.tile([128, 8], f32, tag="ta", bufs=2)
        V.tensor_scalar(ta[:], ea[:], zrec1[:, 0:1], None, ALU.mult)
        V.tensor_scalar(ta[:], ta[:], scol(b, S_GAG), None, ALU.mult)
        tb = psc.tile([128, 8], f32, tag="tb", bufs=2)
        V.tensor_scalar(tb[:], ew[:], zrec1[:, 1:2], None, ALU.mult)
        V.tensor_scalar(tb[:], tb[:], scol(b, S_GAG2), None, ALU.mult)
        V.tensor_add(w32b[:], ta[:], tb[:])

        # -- memory erase+write: Mn = M*(1 - w*ev) + w*wv  (bc over k/d) --
        Mn = pmem.tile([128, K*D], f32, tag="mn", name=f"mn{b}")
        WEXP = psc.tile([128, K*D], f32, tag="wexp", bufs=2)
        for k in range(K):
            V.tensor_scalar(WEXP[:, k*D:(k+1)*D], ones[:, 0:D],
                            w32b[:, k:k+1], None, ALU.mult)
        ev_bc = M(ROWR[b], 0, [[128, 128], [0, K], [1, D]])
        wv_bc = M(ROWR[b], 64, [[128, 128], [0, K], [1, D]])
        mt1 = psc.tile([128, K*D], f32, tag="mt1", bufs=2)
        mview = lambda t: M(t, 0, [[K*D, 128], [D, K], [1, D]])
        V.tensor_tensor(mview(mt1), mview(WEXP), ev_bc, ALU.mult)
        V.tensor_tensor(mview(mt1), mview(Mb), mview(mt1), ALU.mult)
        V.tensor_tensor(mview(Mn), mview(Mb), mview(mt1), ALU.subtract)
        V.tensor_tensor(mview(mt1), mview(WEXP), wv_bc, ALU.mult)
        V.tensor_tensor(mview(Mn), mview(Mn), mview(mt1), ALU.add)
        MnTS = pmem.tile([128, C], f32, tag="mnts", name=f"mnts{b}")
        for k in range(K):
            ps = pet(Mn[:, k*D:(k+1)*D], 128, D)
            V.tensor_copy(MnTS[0:64, k*128:(k+1)*128], ps[:])
            S.square(M(MnTS, 64*C + k*128, [[C, 64], [1, 128]]), ps[:])
        # -- read-content proj+norm --
        RPN = pw.tile([128, 8*K], f32, tag="rpn", bufs=2)  # col = t*32 + r*8 + k
        for k in range(K):
            psR = psmall.tile([128, 8], f32, tag="sp")
            T.matmul(psR[:], MnTS[:, k*128:(k+1)*128],
                     RHSR[:, b*8:(b+1)*8], start=True, stop=True)
            V.tensor_copy(M(RPN, k, [[64, 128], [32, 2], [8, 4]]), psR[:])
        # cosine-r per r: e_r = exp(sp_rs_r * proj/(knr_r*mn+eps))
        PARTS2 = pw.tile([128, 4], f32, tag="parts2", bufs=2)
        RC = pw.tile([128, 32], f32, tag=f"rc{b}")  # col = r*8 + k
        mnr = psc.tile([128, 32], f32, tag="mnr", bufs=2)
        S.activation(mnr[:], RPN[:, 32:64], AF.Ln)
        S.activation(mnr[:], mnr[:], AF.Exp, scale=0.5)
        for r in range(R):
            V.tensor_scalar(mnr[:, r*8:(r+1)*8], mnr[:, r*8:(r+1)*8],
                            scol(b, S_KNR+r), EPS, ALU.mult, ALU.add)
        dnrr = psc.tile([128, 32], f32, tag="dnrr", bufs=2)
        V.reciprocal(dnrr[:], mnr[:])
        V.tensor_mul(dnrr[:], RPN[:, 0:32], dnrr[:])
        for r in range(R):
            S.activation(RC[:, r*8:(r+1)*8], dnrr[:, r*8:(r+1)*8], AF.Exp,
                         scale=scol(b, S_RS+r), accum_out=PARTS2[:, r:r+1])
        psZ2 = psmall.tile([128, 4], f32, tag="sp")
        T.matmul(psZ2[:], ones[:, 0:128], PARTS2[:], start=True, stop=True)
        zrec2 = psc.tile([128, 4], f32, tag="zrec2", bufs=2)
        V.reciprocal(zrec2[:], psZ2[:])
        for r in range(R):
            V.tensor_scalar(RC[:, r*8:(r+1)*8], RC[:, r*8:(r+1)*8],
                            zrec2[:, r:r+1], None, ALU.mult)

        # -- X8 lhsT: col k*8 + [rw(0:4) | w*rw(4:8)] --
        X8 = pw.tile([128, K*8], f32r, tag="x8", bufs=2)
        for r in range(R):
            S.copy(M(X8, r, [[K*8, 128], [8, K]]),
                   rwT[:, r*32+b*8:r*32+(b+1)*8])
            V.tensor_mul(M(X8, 4+r, [[K*8, 128], [8, K]]),
                         rwT[:, r*32+b*8:r*32+(b+1)*8], w32b[:])
        # w-dots partials
        for r in range(R):
            scr = psc.tile([128, 8], f32, tag="scr", bufs=2)
            V.tensor_mul(scr[:], rwT[:, r*32+b*8:r*32+(b+1)*8], w32b[:])
            V.tensor_reduce(PARTSD[:, b*8+4+r:b*8+4+r+1], scr[:], AX.X, ALU.add)
        psD = psmall.tile([128, 8], f32, tag="sp")
        T.matmul(psD[:], ones[:, 0:128], PARTSD[:, b*8:(b+1)*8],
                 start=True, stop=True)
        DOTS = pw.tile([128, 8], f32, tag="dots", bufs=2)  # [pdots r | wdots r]
        V.tensor_copy(DOTS[:], psD[:])

        # -- link matmuls: bwd on LN, fwd on LT (f32r) --
        psB = [pacc.tile([8, 512], f32, tag=f"acc{h}", name=f"psB{h}")
               for h in range(2)]
        psF = [pacc.tile([8, 512], f32, tag=f"acc{h}", name=f"psF{h}")
               for h in range(2)]
        for ih in range(K):
            lw = X8[:, ih*8:(ih+1)*8]
            for h in range(2):
                T.matmul(psB[h][:], lw,
                         LN[ih][:, h*512:(h+1)*512],
                         start=(ih == 0), stop=(ih == K-1))
        for jh in range(K):
            lw = X8[:, jh*8:(jh+1)*8]
            for h in range(2):
                T.matmul(psF[h][:], lw,
                         LT[jh][:, h*512:(h+1)*512],
                         start=(jh == 0), stop=(jh == K-1))
        brow = pw.tile([8, C], f32, tag="brow", bufs=2)
        frow = pw.tile([8, C], f32, tag="frow", bufs=2)
        for h in range(2):
            S.copy(brow[:, h*512:(h+1)*512], psB[h][:])
            V.tensor_copy(frow[:, h*512:(h+1)*512], psF[h][:])

        # -- transpose link products to cP: [128, 32] (r,k) tiles --
        BM = pw.tile([128, 32], f32, tag="bm", bufs=2)   # rwL - wrwL
        BV = pw.tile([128, 32], f32, tag="bv", bufs=2)   # rwL
        FM = pw.tile([128, 32], f32, tag="fm", bufs=2)   # Lrw
        FV = pw.tile([128, 32], f32, tag="fv", bufs=2)   # Lwrw
        for k in range(K):
            ps = pet(brow[:, k*128:(k+1)*128], 8, 128)   # [128, 8]
            scb = psc.tile([128, 8], f32, tag="tbk", bufs=2)
            S.copy(scb[:], ps[:])
            V.tensor_sub(M(BM, k, [[32, 128], [8, 4]]), scb[:, 0:4], scb[:, 4:8])
            V.tensor_copy(M(BV, k, [[32, 128], [8, 4]]), scb[:, 0:4])
            ps = pet(frow[:, k*128:(k+1)*128], 8, 128)
            scf = psc.tile([128, 8], f32, tag="tfk", bufs=2)
            S.copy(scf[:], ps[:])
            V.tensor_copy(M(FM, k, [[32, 128], [8, 4]]), scf[:, 0:4])
            V.tensor_copy(M(FV, k, [[32, 128], [8, 4]]), scf[:, 4:8])

        # -- corrections -> raw directional weights (cP, col r*8+k) --
        w_bcr = M(w32b, 0, [[8, 128], [0, 4], [1, 8]])
        p_bcr = M(p32, b*8, [[32, 128], [0, 4], [1, 8]])
        rw_brk = M(rwT, b*8, [[128, 128], [32, 4], [1, 8]])
        pdots_bc = M(DOTS, 0, [[8, 128], [1, 4], [0, 8]])
        wdots_bc = M(DOTS, 4, [[8, 128], [1, 4], [0, 8]])
        WP8 = psc.tile([128, 8], f32, tag="wp8", bufs=2)
        V.tensor_mul(WP8[:], w32b[:], p32[:, b*8:(b+1)*8])
        DT = psc.tile([128, 32], f32, tag="dt", bufs=2)
        V.tensor_tensor(DT[:], rw_brk, M(WP8, 0, [[8, 128], [0, 4], [1, 8]]),
                        ALU.mult)
        SC1 = psc.tile([128, 32], f32, tag="sc1", bufs=2)
        SC2 = psc.tile([128, 32], f32, tag="sc2", bufs=2)
        BRAW = pw.tile([128, 32], f32, tag="braw", bufs=2)
        V.tensor_tensor(SC1[:], BV[:], w_bcr, ALU.mult)
        V.tensor_sub(SC2[:], BM[:], SC1[:])
        for r in range(R):
            V.tensor_scalar(SC1[:, r*8:(r+1)*8], p32[:, b*8:(b+1)*8],
                            DOTS[:, 4+r:5+r], None, ALU.mult)
        V.tensor_add(SC2[:], SC2[:], SC1[:])
        V.tensor_sub(BRAW[:], SC2[:], DT[:])
        FRAW = pw.tile([128, 32], f32, tag="fraw", bufs=2)
        V.tensor_tensor(SC1[:], FM[:], w_bcr, ALU.mult)
        V.tensor_sub(SC2[:], FM[:], SC1[:])
        V.tensor_sub(SC2[:], SC2[:], FV[:])
        for r in range(R):
            V.tensor_scalar(SC1[:, r*8:(r+1)*8], w32b[:],
                            DOTS[:, r:r+1], None, ALU.mult)
        V.tensor_add(SC2[:], SC2[:], SC1[:])
        V.tensor_sub(FRAW[:], SC2[:], DT[:])

        # -- sharpen both directions --
        ms_bc = M(SREP[b], S_MS, [[NS, 128], [1, 4], [0, 8]])
        PARTS3 = pw.tile([128, 8], f32, tag="parts3", bufs=2)
        sharp = []
        for di, raw in enumerate((BRAW, FRAW)):
            SH = pw.tile([128, 32], f32, tag=f"sh{di}")
            V.tensor_scalar(SH[:], raw[:], 0.0, None, ALU.max)
            S.activation(SH[:], SH[:], AF.Ln, bias=epsc[:])
            for r in range(R):
                V.tensor_scalar(SH[:, r*8:(r+1)*8], SH[:, r*8:(r+1)*8],
                                scol(b, S_MS+r), None, ALU.mult)
            for r in range(R):
                S.activation(SH[:, r*8:(r+1)*8], SH[:, r*8:(r+1)*8], AF.Exp,
                             accum_out=PARTS3[:, di*4+r:di*4+r+1])
            sharp.append(SH)
        psZ3 = psmall.tile([128, 8], f32, tag="sp")
        T.matmul(psZ3[:], ones[:, 0:128], PARTS3[:], start=True, stop=True)
        zrec3 = psc.tile([128, 8], f32, tag="zrec3", bufs=2)
        V.reciprocal(zrec3[:], psZ3[:])
        for di in range(2):
            for r in range(R):
                V.tensor_scalar(sharp[di][:, r*8:(r+1)*8],
                                sharp[di][:, r*8:(r+1)*8],
                                zrec3[:, di*4+r:di*4+r+1], None, ALU.mult)

        # -- mode mixing + read vectors --
        def pie_bc(m):
            return M(SREP[b], S_PIE+m, [[NS, 128], [3, 4], [0, 8]])
        MX1 = psc.tile([128, 32], f32, tag="mx1", bufs=2)
        MX2 = psc.tile([128, 32], f32, tag="mx2", bufs=2)
        RWTS = pw.tile([128, 32], f32, tag="rwts", bufs=2)
        for r in range(R):
            sl = slice(r*8, (r+1)*8)
            V.tensor_scalar(MX1[:, sl], RC[:, sl],
                            scol(b, S_PIE + r*3 + 1), None, ALU.mult)
            V.tensor_scalar(MX2[:, sl], sharp[1][:, sl],
                            scol(b, S_PIE + r*3 + 2), None, ALU.mult)
        V.tensor_add(MX1[:], MX1[:], MX2[:])
        for r in range(R):
            sl = slice(r*8, (r+1)*8)
            V.tensor_scalar(MX2[:, sl], sharp[0][:, sl],
                            scol(b, S_PIE + r*3 + 0), None, ALU.mult)
        V.tensor_add(RWTS[:], MX1[:], MX2[:])
        psRV = psmall.tile([BB, D], f32, tag="sp")
        for k in range(K):
            T.matmul(psRV[:], M(RWTS, k, [[32, 128], [8, 4]]),
                     Mn[:, k*D:(k+1)*D], start=(k == 0), stop=(k == K-1))
        rv_sb = pw.tile([BB, D], f32, tag="rvsb", bufs=2)
        S.copy(rv_sb[:], psRV[:])
        SY.dma_start(out_rv[b], rv_sb[:])


# ======================= host-side wrapper =======================
_CACHE = {}

def _get_program():
    if "nc" not in _CACHE:
        _CACHE["nc"] = build_program()
    return _CACHE["nc"]


def kernel(x, W_if, b_if, memory, usage, link, precedence,
           prev_read_weights, prev_write_weights):
    from concourse.bass_utils import run_bass_kernel_spmd
    nc = _get_program()
    f4 = np.float32
    eye = np.eye(128, dtype=f4)
    one = np.ones((128, 128), dtype=f4)
    selb = np.zeros((BB, BB * 128), f4)
    for bb in range(BB):
        selb[bb, bb*128:(bb+1)*128] = 1.0
    in_maps = []
    for c in range(NC):
        s = slice(c*BB, (c+1)*BB)
        in_maps.append({
            "x": np.ascontiguousarray(x[s], f4),
            "W_if": np.ascontiguousarray(W_if, f4),
            "b_if": np.ascontiguousarray(b_if, f4),
            "memory": np.ascontiguousarray(memory[s], f4),
            "usage": np.ascontiguousarray(usage[s], f4),
            "link": np.ascontiguousarray(link[s].reshape(BB, C, C), f4),
            "precedence": np.ascontiguousarray(precedence[s].reshape(BB, C), f4),
            "prw": np.ascontiguousarray(prev_read_weights[s], f4),
            "pww": np.ascontiguousarray(prev_write_weights[s].reshape(BB, C), f4),
            "eye128": eye,
            "ones128": one,
            "selb": selb,
            "eyer": eye,
        })
    res = run_bass_kernel_spmd(nc, in_maps, list(range(NC)))
    out = np.concatenate([res.results[c]["read_vectors"] for c in range(NC)],
                         axis=0)
    return out

